# revision 15
# baseline (speedup 1.0000x reference)
"""Transformer encoder layer (B=4, S=2048, D=1024, H=16, FFN=4096) on 8 TRN2 cores.

Sharding: core c owns batch b=c//2, sequence half h=c%2 (1024 query tokens).
Each core computes full-sequence K/V for its batch element (no collectives).
All matmuls run in float32r (full PE rate, ~1.5e-4 rel err).

Self-contained: hardcodes shapes; builds one SPMD Bass program and runs it
via run_bass_kernel_spmd on cores 0-7.
"""
from contextlib import ExitStack

import numpy as np

import concourse.bass as bass
import concourse.tile as tile
from concourse import bacc, mybir
from concourse.bass_utils import run_bass_kernel_spmd
from concourse.masks import make_identity

F32 = mybir.dt.float32
F32R = mybir.dt.float32r

B, S, D, H, DH, HID = 4, 2048, 1024, 16, 64, 4096
SQ = S // 2           # query tokens per core
N_CORES = 8
LN_EPS = 1e-5
SCALE = 1.0 / np.sqrt(DH)

KO = D // 128         # 8   contraction subtiles over D
KT = S // 128         # 16  key-token tiles
QT = SQ // 128        # 8   query-token tiles
HP = H // 2           # 8   head pairs
HT = HID // 128       # 32  hidden tiles
HB = 4                # hidden blocks (of 8 ht = 1024 hid each)

_BUILD_CACHE = {}


def _build(flags, debug=None):
    """flags: frozenset of extras: bqkv, bo, b1, b2, g1b1, g2b2.
    debug: None | 'ctx' | 'y' (adds early outputs and stops there)."""
    nc = bacc.Bacc("TRN2", target_bir_lowering=False, debug=False)

    XT = nc.dram_tensor("XT", [D, S], F32R, kind="ExternalInput").ap()
    XQT = nc.dram_tensor("XQT", [D, SQ], F32R, kind="ExternalInput").ap()
    XQ = nc.dram_tensor("XQ", [SQ, D], F32, kind="ExternalInput").ap()
    WQ = nc.dram_tensor("WQ", [D, D], F32R, kind="ExternalInput").ap()
    WK = nc.dram_tensor("WK", [D, D], F32R, kind="ExternalInput").ap()
    WV = nc.dram_tensor("WV", [D, D], F32R, kind="ExternalInput").ap()
    WO = nc.dram_tensor("WO", [D, D], F32R, kind="ExternalInput").ap()
    W1 = nc.dram_tensor("W1", [D, HID], F32R, kind="ExternalInput").ap()
    W2 = nc.dram_tensor("W2", [HID, D], F32R, kind="ExternalInput").ap()
    OUT = nc.dram_tensor("OUT", [SQ, D], F32, kind="ExternalOutput").ap()
    if debug == "ctx":
        CTXD = nc.dram_tensor("CTXD", [128, HP, SQ], F32, kind="ExternalOutput").ap()
    if debug == "y":
        YD = nc.dram_tensor("YD", [QT, 128, D], F32, kind="ExternalOutput").ap()

    ext = {}
    if "bqkv" in flags:
        for nm in ("BQ", "BK", "BV"):
            ext[nm] = nc.dram_tensor(nm, [D], F32, kind="ExternalInput").ap()
    if "bo" in flags:
        ext["BO"] = nc.dram_tensor("BO", [D], F32, kind="ExternalInput").ap()
    if "b1" in flags:
        ext["B1"] = nc.dram_tensor("B1", [HID], F32, kind="ExternalInput").ap()
    if "b2" in flags:
        ext["B2"] = nc.dram_tensor("B2", [D], F32, kind="ExternalInput").ap()
    if "g1b1" in flags:
        ext["G1"] = nc.dram_tensor("G1", [D], F32, kind="ExternalInput").ap()
        ext["BT1"] = nc.dram_tensor("BT1", [D], F32, kind="ExternalInput").ap()
    if "g2b2" in flags:
        ext["G2"] = nc.dram_tensor("G2", [D], F32, kind="ExternalInput").ap()
        ext["BT2"] = nc.dram_tensor("BT2", [D], F32, kind="ExternalInput").ap()

    def bcast_free(vec_ap, parts):
        """1-D DRAM AP -> DMA source AP broadcast over `parts` partitions."""
        return bass.AP(tensor=vec_ap.tensor, offset=vec_ap.offset,
                       ap=[[0, parts]] + list(vec_ap.ap))

    WKr = WK.rearrange("(ko p) d -> p ko d", p=128)
    WQr = WQ.rearrange("(ko p) d -> p ko d", p=128)
    WVr = WV.rearrange("(ko p) d -> p ko d", p=128)
    WOr = WO.rearrange("(ko p) d -> p ko d", p=128)
    W1r = W1.rearrange("(ko p) h -> p ko h", p=128)
    W2r = W2.rearrange("(ho p) d -> p ho d", p=128)

    with tile.TileContext(nc) as tc, ExitStack() as ctx:
        persist = ctx.enter_context(tc.tile_pool(name="persist", bufs=1))
        dram = ctx.enter_context(tc.tile_pool(name="dram", bufs=1, space="DRAM"))

        Vd = dram.tile([KT, 128, H, DH + 1], F32R)
        KTd = dram.tile([HP, 128, S], F32R)
        QTd = dram.tile([HP, 128, SQ], F32R)
        Yd = dram.tile([QT, 128, D], F32)

        stk_x = ctx.enter_context(ExitStack())
        pX = stk_x.enter_context(tc.tile_pool(name="pX", bufs=1))
        xt = pX.tile([128, KO, S], F32R)
        nc.sync.dma_start(xt[:], XT.rearrange("(ko p) t -> p ko t", p=128))
        xqt = pX.tile([128, KO, SQ], F32R)
        nc.sync.dma_start(xqt[:], XQT.rearrange("(ko p) t -> p ko t", p=128))

        ones16_f = persist.tile([128, 16], F32)
        nc.vector.memset(ones16_f[:], 1.0)
        ones16 = persist.tile([128, 16], F32R)
        nc.scalar.copy(ones16[:], ones16_f[:])
        ones64_f = persist.tile([128, 64], F32)
        nc.vector.memset(ones64_f[:], 1.0)
        ones64 = persist.tile([128, 64], F32R)
        nc.scalar.copy(ones64[:], ones64_f[:])
        eps_sb = persist.tile([128, 1], F32)
        nc.vector.memset(eps_sb[:], LN_EPS)
        ident = persist.tile([128, 128], F32)
        make_identity(nc, ident[:])

        if "bqkv" in flags:
            bq_sb = persist.tile([128, KO], F32)
            bk_sb = persist.tile([128, KO], F32)
            nc.sync.dma_start(bq_sb[:], ext["BQ"].rearrange("(o p) -> p o", p=128))
            nc.sync.dma_start(bk_sb[:], ext["BK"].rearrange("(o p) -> p o", p=128))
        if "b1" in flags:
            b1_sb = persist.tile([128, HT], F32)
            nc.sync.dma_start(b1_sb[:], ext["B1"].rearrange("(o p) -> p o", p=128))
        if "bo" in flags:
            bo_sb = persist.tile([128, D], F32)
            nc.sync.dma_start(bo_sb[:], bcast_free(ext["BO"], 128))
        if "b2" in flags:
            b2_sb = persist.tile([128, D], F32)
            nc.sync.dma_start(b2_sb[:], bcast_free(ext["B2"], 128))
        if "g1b1" in flags:
            g1_sb = persist.tile([128, D], F32)
            bt1_sb = persist.tile([128, D], F32)
            nc.sync.dma_start(g1_sb[:], bcast_free(ext["G1"], 128))
            nc.sync.dma_start(bt1_sb[:], bcast_free(ext["BT1"], 128))
        if "g2b2" in flags:
            g2_sb = persist.tile([128, D], F32)
            bt2_sb = persist.tile([128, D], F32)
            nc.sync.dma_start(g2_sb[:], bcast_free(ext["G2"], 128))
            nc.sync.dma_start(bt2_sb[:], bcast_free(ext["BT2"], 128))

        # ---------- Phase A1: V projection -> Vd ----------
        with (
            tc.tile_pool(name="pA_w", bufs=2) as paw,
            tc.tile_pool(name="pA_s", bufs=3) as pas,
            tc.tile_pool(name="psA", bufs=3, space="PSUM") as psa,
        ):
            for dhalf in range(2):
                wv_h = paw.tile([128, KO, 512], F32R, tag="wv")
                nc.sync.dma_start(
                    wv_h[:], WVr[:, :, dhalf * 512:(dhalf + 1) * 512])
                for tt in range(KT):
                    pvp = psa.tile([128, 512], F32)
                    for k in range(KO):
                        nc.tensor.matmul(
                            pvp[:], xt[:, k, tt * 128:(tt + 1) * 128],
                            wv_h[:, k], start=(k == 0), stop=(k == KO - 1))
                    vstage = pas.tile([128, 8, DH], F32R, tag="vs")
                    vsv = vstage[:].rearrange("p a b -> p (a b)")
                    if "bqkv" in flags:
                        bvb = pas.tile([128, 512], F32, tag="bv")
                        nc.sync.dma_start(
                            bvb[:], bcast_free(
                                ext["BV"][dhalf * 512:(dhalf + 1) * 512], 128))
                        nc.vector.tensor_add(vsv, pvp[:], bvb[:])
                    else:
                        nc.scalar.copy(vsv, pvp[:])
                    nc.sync.dma_start(
                        Vd[tt, :, dhalf * 8:(dhalf + 1) * 8, 0:DH], vstage[:])
            for tt in range(KT):
                nc.sync.dma_start(Vd[tt, :, :, DH:DH + 1], ones16[:, :, None])

        # ---------- Phase A2: K^T / Q^T projections -> KTd / QTd ----------
        with (
            tc.tile_pool(name="pA2_w", bufs=2) as pa2w,
            tc.tile_pool(name="pA2_s", bufs=3) as pa2s,
            tc.tile_pool(name="psA2", bufs=3, space="PSUM") as psa2,
        ):
            for hp in range(HP):
                wk_hp = pa2w.tile([128, KO, 128], F32R, tag="wk")
                nc.sync.dma_start(wk_hp[:], WKr[:, :, hp * 128:(hp + 1) * 128])
                wq_hp = pa2w.tile([128, KO, 128], F32R, tag="wq")
                nc.sync.dma_start(wq_hp[:], WQr[:, :, hp * 128:(hp + 1) * 128])
                for ns in range(S // 512):
                    pk = psa2.tile([128, 512], F32)
                    for k in range(KO):
                        nc.tensor.matmul(
                            pk[:], wk_hp[:, k], xt[:, k, ns * 512:(ns + 1) * 512],
                            start=(k == 0), stop=(k == KO - 1))
                    ks = pa2s.tile([128, 512], F32R, tag="ks")
                    if "bqkv" in flags:
                        nc.scalar.activation(
                            ks[:], pk[:], mybir.ActivationFunctionType.Identity,
                            bias=bk_sb[:, hp:hp + 1])
                    else:
                        nc.scalar.copy(ks[:], pk[:])
                    nc.sync.dma_start(
                        KTd[hp, :, ns * 512:(ns + 1) * 512], ks[:])
                for ns in range(SQ // 512):
                    pq = psa2.tile([128, 512], F32)
                    for k in range(KO):
                        nc.tensor.matmul(
                            pq[:], wq_hp[:, k], xqt[:, k, ns * 512:(ns + 1) * 512],
                            start=(k == 0), stop=(k == KO - 1))
                    qs = pa2s.tile([128, 512], F32R, tag="qs")
                    if "bqkv" in flags:
                        nc.scalar.activation(
                            qs[:], pq[:], mybir.ActivationFunctionType.Identity,
                            bias=bq_sb[:, hp:hp + 1])
                    else:
                        nc.scalar.copy(qs[:], pq[:])
                    nc.sync.dma_start(
                        QTd[hp, :, ns * 512:(ns + 1) * 512], qs[:])

        stk_x.close()  # free xt/xqt

        # ---------- Phase B: attention per head-pair ----------
        # pYT opened before pCTX so closes can follow stack order
        # (pCTX closes after phase C, pYT at the end).
        stk_yt = ctx.enter_context(ExitStack())
        pYT = stk_yt.enter_context(tc.tile_pool(name="pYT", bufs=1))
        stk_ctx = ctx.enter_context(ExitStack())
        pCTX = stk_ctx.enter_context(tc.tile_pool(name="pCTX", bufs=1))
        ctxT = pCTX.tile([128, HP, SQ], F32R)
        with (
            tc.tile_pool(name="pB_kq", bufs=2) as pbkq,
            tc.tile_pool(name="pB_v", bufs=4) as pbv,
            tc.tile_pool(name="pB_p", bufs=6) as pbp,
            tc.tile_pool(name="pB_n", bufs=2) as pbn,
            tc.tile_pool(name="psB", bufs=3, space="PSUM") as psb,
            tc.tile_pool(name="psPV", bufs=4, space="PSUM") as pspv,
            tc.tile_pool(name="psBC", bufs=1, space="PSUM") as psbc,
        ):
            for hp in range(HP):
                kt_hp = pbkq.tile([128, S], F32R, tag="kt")
                nc.sync.dma_start(kt_hp[:], KTd[hp])
                qt_hp = pbkq.tile([128, SQ], F32R, tag="qt")
                nc.sync.dma_start(qt_hp[:], QTd[hp])

                pv_ps = [[pspv.tile([128, 512], F32, tag="pv",
                                    name=f"pv_{hp}_{h}_{qb}")
                          for qb in range(2)]
                         for h in range(2)]  # [h][qb]
                for kt in range(KT):
                    v_kt = pbv.tile([128, 2, DH + 1], F32R, tag="v")
                    nc.sync.dma_start(v_kt[:], Vd[kt, :, 2 * hp:2 * hp + 2, :])
                    for qb in range(2):
                        for h in range(2):
                            ps_s = psb.tile([128, 512], F32, tag="ps_s")
                            nc.tensor.matmul(
                                ps_s[:],
                                kt_hp[h * 64:(h + 1) * 64, kt * 128:(kt + 1) * 128],
                                qt_hp[h * 64:(h + 1) * 64, qb * 512:(qb + 1) * 512],
                                start=True, stop=True)
                            p_t = pbp.tile([128, 512], F32R, tag="p")
                            nc.scalar.activation(
                                p_t[:], ps_s[:], mybir.ActivationFunctionType.Exp,
                                bias=0.0, scale=float(SCALE))
                            nc.tensor.matmul(
                                pv_ps[h][qb][0:DH + 1], v_kt[:, h], p_t[:],
                                start=(kt == 0), stop=(kt == KT - 1),
                                skip_group_check=True)
                # normalization: broadcast sumexp across 64 partitions via a
                # ones-column matmul, reciprocal, then scale ctx rows
                for qb in range(2):
                    for h in range(2):
                        se = pbn.tile([128, 512], F32R, tag="se",
                                      name=f"se_{hp}_{h}_{qb}")
                        nc.scalar.copy(se[64:65, :], pv_ps[h][qb][DH:DH + 1, :])
                        bc = psbc.tile([64, 512], F32, tag="bc",
                                       name=f"bc_{hp}_{h}_{qb}")
                        nc.tensor.matmul(bc[:], ones64[64:65, :], se[64:65, :],
                                         start=True, stop=True)
                        rb = pbn.tile([64, 512], F32, tag="rb",
                                      name=f"rb_{hp}_{h}_{qb}")
                        nc.vector.reciprocal(rb[:], bc[:])
                        if h == 0:
                            nc.vector.tensor_mul(
                                ctxT[0:64, hp, qb * 512:(qb + 1) * 512],
                                pv_ps[0][qb][0:DH], rb[:])
                        else:
                            tmp1 = pbn.tile([64, 512], F32R, tag="tmp1",
                                            name=f"tmp1_{hp}_{qb}")
                            nc.vector.tensor_mul(tmp1[:], pv_ps[1][qb][0:DH],
                                                 rb[:])
                            nc.sync.dma_start(
                                ctxT[64:128, hp, qb * 512:(qb + 1) * 512],
                                tmp1[:])

        if debug == "ctx":
            with tc.tile_pool(name="dbg", bufs=2) as dbg:
                for hp in range(HP):
                    t = dbg.tile([128, SQ], F32)
                    nc.vector.tensor_copy(t[:], ctxT[:, hp, :])
                    nc.sync.dma_start(CTXD[:, hp, :], t[:])
            stk_ctx.close()
            out_stub = persist.tile([128, 1], F32)
            nc.vector.memset(out_stub[:], 0.0)
            nc.sync.dma_start(OUT[0:1, 0:128].rearrange("a b -> b a"), out_stub[:])

        phase_cd = debug != "ctx"
        phase_d = debug is None

        # ---------- Phase C: out-proj + LN1 + transpose ----------
        yt = pYT.tile([128, KO, SQ], F32R, name="yt") if phase_cd else None
        with (
            tc.tile_pool(name="pC_w", bufs=1) as pcw,
            tc.tile_pool(name="pC_x", bufs=1) as pcx,
            tc.tile_pool(name="pC_s", bufs=3) as pcs,
            tc.tile_pool(name="psC", bufs=3, space="PSUM") as psc,
            tc.tile_pool(name="psT", bufs=3, space="PSUM") as pst,
        ):
          if phase_cd:
            wo = pcw.tile([128, KO, D], F32R)
            nc.sync.dma_start(wo[:], WOr)
            xq = pcx.tile([128, QT, D], F32)
            nc.sync.dma_start(xq[:], XQ.rearrange("(qt p) d -> p qt d", p=128))
            for qt in range(QT):
                r1 = pcs.tile([128, D], F32, tag="r1")
                for dh in range(2):
                    po = psc.tile([128, 512], F32)
                    for hp in range(HP):
                        nc.tensor.matmul(
                            po[:], ctxT[:, hp, qt * 128:(qt + 1) * 128],
                            wo[:, hp, dh * 512:(dh + 1) * 512],
                            start=(hp == 0), stop=(hp == HP - 1))
                    nc.vector.tensor_add(
                        r1[:, dh * 512:(dh + 1) * 512], po[:],
                        xq[:, qt, dh * 512:(dh + 1) * 512])
                if "bo" in flags:
                    nc.vector.tensor_add(r1[:], r1[:], bo_sb[:])
                stats = pcs.tile([128, 2, 6], F32, tag="st")
                r1v = r1[:].rearrange("p (s d) -> p s d", s=2)
                for sgi in range(2):
                    nc.vector.bn_stats(stats[:, sgi], r1v[:, sgi])
                mv = pcs.tile([128, 2], F32, tag="mv")
                nc.vector.bn_aggr(mv[:], stats[:])
                rstd = pcs.tile([128, 1], F32, tag="rstd")
                nc.scalar.activation(rstd[:], mv[:, 1:2],
                                     mybir.ActivationFunctionType.Sqrt,
                                     bias=eps_sb[:], scale=1.0)
                nc.vector.reciprocal(rstd[:], rstd[:])
                ytile = pcs.tile([128, D], F32, tag="ytile")
                nc.vector.tensor_scalar(
                    ytile[:], r1[:], scalar1=mv[:, 0:1], scalar2=rstd[:],
                    op0=mybir.AluOpType.subtract, op1=mybir.AluOpType.mult)
                if "g1b1" in flags:
                    nc.vector.tensor_mul(ytile[:], ytile[:], g1_sb[:])
                    nc.vector.tensor_add(ytile[:], ytile[:], bt1_sb[:])
                nc.sync.dma_start(Yd[qt], ytile[:])
                for dt in range(KO):
                    ptp = pst.tile([128, 128], F32)
                    nc.tensor.transpose(
                        ptp[:], ytile[:, dt * 128:(dt + 1) * 128], ident[:])
                    nc.vector.tensor_copy(
                        yt[:, dt, qt * 128:(qt + 1) * 128], ptp[:])

        if phase_cd:
            stk_ctx.close()  # free ctxT

        if debug == "y":
            with tc.tile_pool(name="dbg2", bufs=2) as dbg2:
                for qt in range(QT):
                    t = dbg2.tile([128, D], F32)
                    nc.sync.dma_start(t[:], Yd[qt])
                    nc.sync.dma_start(YD[qt], t[:])
            out_stub2 = persist.tile([128, 1], F32)
            nc.vector.memset(out_stub2[:], 0.0)
            nc.sync.dma_start(OUT[0:1, 0:128].rearrange("a b -> b a"), out_stub2[:])

        # ---------- Phase D: FFN + LN2 + output ----------
        with (
            tc.tile_pool(name="pD_w1", bufs=1) as pw1,
            tc.tile_pool(name="pD_w2", bufs=1) as pw2,
            tc.tile_pool(name="pD_ft", bufs=1) as pft,
            tc.tile_pool(name="pD_acc", bufs=1) as pacc,
            tc.tile_pool(name="pD_s", bufs=2) as pds,
            tc.tile_pool(name="psD", bufs=3, space="PSUM") as psd,
            tc.tile_pool(name="psD2", bufs=4, space="PSUM") as psd2,
        ):
          if phase_d:
            acc = pacc.tile([128, QT, D], F32)
            for hb in range(HB):
                w1_hb = pw1.tile([128, KO, 1024], F32R)
                nc.sync.dma_start(
                    w1_hb[:], W1r[:, :, hb * 1024:(hb + 1) * 1024])
                w2_hb = pw2.tile([128, 8, D], F32R)
                nc.sync.dma_start(w2_hb[:], W2r[:, hb * 8:(hb + 1) * 8, :])
                ft = pft.tile([128, 8, SQ], F32R)
                for hti in range(8):
                    for qb in range(2):
                        pf = psd.tile([128, 512], F32)
                        for k in range(KO):
                            nc.tensor.matmul(
                                pf[:], w1_hb[:, k, hti * 128:(hti + 1) * 128],
                                yt[:, k, qb * 512:(qb + 1) * 512],
                                start=(k == 0), stop=(k == KO - 1))
                        if "b1" in flags:
                            nc.scalar.activation(
                                ft[:, hti, qb * 512:(qb + 1) * 512], pf[:],
                                mybir.ActivationFunctionType.Relu,
                                bias=b1_sb[:, hb * 8 + hti:hb * 8 + hti + 1])
                        else:
                            nc.scalar.activation(
                                ft[:, hti, qb * 512:(qb + 1) * 512], pf[:],
                                mybir.ActivationFunctionType.Relu)
                for qt in range(QT):
                    for dh in range(2):
                        p2 = psd2.tile([128, 512], F32)
                        for hti in range(8):
                            nc.tensor.matmul(
                                p2[:], ft[:, hti, qt * 128:(qt + 1) * 128],
                                w2_hb[:, hti, dh * 512:(dh + 1) * 512],
                                start=(hti == 0), stop=(hti == 7))
                        if hb == 0:
                            nc.vector.tensor_copy(
                                acc[:, qt, dh * 512:(dh + 1) * 512], p2[:])
                        else:
                            nc.vector.tensor_add(
                                acc[:, qt, dh * 512:(dh + 1) * 512],
                                acc[:, qt, dh * 512:(dh + 1) * 512], p2[:])
            # residual + LN2 + store
            for qt in range(QT):
                yr = pds.tile([128, D], F32, tag="yr")
                nc.sync.dma_start(yr[:], Yd[qt])
                r2 = pds.tile([128, D], F32, tag="r2")
                nc.vector.tensor_add(r2[:], acc[:, qt, :], yr[:])
                if "b2" in flags:
                    nc.vector.tensor_add(r2[:], r2[:], b2_sb[:])
                stats = pds.tile([128, 2, 6], F32, tag="st2")
                r2v = r2[:].rearrange("p (s d) -> p s d", s=2)
                for sgi in range(2):
                    nc.vector.bn_stats(stats[:, sgi], r2v[:, sgi])
                mv = pds.tile([128, 2], F32, tag="mv2")
                nc.vector.bn_aggr(mv[:], stats[:])
                rstd = pds.tile([128, 1], F32, tag="rstd2")
                nc.scalar.activation(rstd[:], mv[:, 1:2],
                                     mybir.ActivationFunctionType.Sqrt,
                                     bias=eps_sb[:], scale=1.0)
                nc.vector.reciprocal(rstd[:], rstd[:])
                o = pds.tile([128, D], F32, tag="o")
                nc.vector.tensor_scalar(
                    o[:], r2[:], scalar1=mv[:, 0:1], scalar2=rstd[:],
                    op0=mybir.AluOpType.subtract, op1=mybir.AluOpType.mult)
                if "g2b2" in flags:
                    nc.vector.tensor_mul(o[:], o[:], g2_sb[:])
                    nc.vector.tensor_add(o[:], o[:], bt2_sb[:])
                nc.sync.dma_start(
                    OUT.rearrange("(qt p) d -> qt p d", p=128)[qt], o[:])

    nc.compile()
    return nc


def _get_program(flags, debug=None):
    key = (flags, debug)
    if key not in _BUILD_CACHE:
        _BUILD_CACHE[key] = _build(flags, debug)
    return _BUILD_CACHE[key]


def _make_in_maps(X, shared):
    in_maps = []
    for c in range(N_CORES):
        b, half = c // 2, c % 2
        xq = np.ascontiguousarray(X[b, half * SQ:(half + 1) * SQ])
        m = dict(shared)
        m.update({"XT": np.ascontiguousarray(X[b].T),
                  "XQT": np.ascontiguousarray(xq.T), "XQ": xq})
        in_maps.append(m)
    return in_maps


def kernel(X, Wq, bq, Wk, bk, Wv, bv, Wo, bo, g1, beta1, W1, b1, W2, b2, g2,
           beta2, _debug=None, _trace=False):
    f32 = lambda a: np.ascontiguousarray(np.asarray(a), dtype=np.float32)
    X = f32(X)
    Wq, Wk, Wv, Wo, W1, W2 = map(f32, (Wq, Wk, Wv, Wo, W1, W2))
    bq, bk, bv, bo, b1, b2 = map(f32, (bq, bk, bv, bo, b1, b2))
    g1, beta1, g2, beta2 = map(f32, (g1, beta1, g2, beta2))

    flags = set()
    if bq.any() or bk.any() or bv.any():
        flags.add("bqkv")
    if bo.any():
        flags.add("bo")
    if b1.any():
        flags.add("b1")
    if b2.any():
        flags.add("b2")
    if (g1 != 1).any() or beta1.any():
        flags.add("g1b1")
    if (g2 != 1).any() or beta2.any():
        flags.add("g2b2")
    flags = frozenset(flags)

    nc = _get_program(flags, _debug)

    shared = {"WQ": Wq, "WK": Wk, "WV": Wv, "WO": Wo, "W1": W1, "W2": W2}
    if "bqkv" in flags:
        shared.update({"BQ": bq, "BK": bk, "BV": bv})
    if "bo" in flags:
        shared["BO"] = bo
    if "b1" in flags:
        shared["B1"] = b1
    if "b2" in flags:
        shared["B2"] = b2
    if "g1b1" in flags:
        shared.update({"G1": g1, "BT1": beta1})
    if "g2b2" in flags:
        shared.update({"G2": g2, "BT2": beta2})

    in_maps = _make_in_maps(X, shared)
    res = run_bass_kernel_spmd(nc, in_maps, core_ids=list(range(N_CORES)),
                               trace=_trace)

    if _debug is not None or _trace:
        return res

    out = np.empty((B, S, D), dtype=np.float32)
    for c in range(N_CORES):
        b, half = c // 2, c % 2
        out[b, half * SQ:(half + 1) * SQ] = res.results[c]["OUT"]
    return out


# revision 26
# speedup vs baseline: 1.0613x; 1.0613x over previous
"""Transformer encoder layer (B=4, S=2048, D=1024, H=16, FFN=4096) on 8 TRN2 cores.

Sharding: core c owns batch b=c//2, sequence half h=c%2 (1024 query tokens).
Each core computes full-sequence K/V for its batch element (no collectives).
All matmuls run in float32r (full PE rate, ~1.5e-4 rel err).

Self-contained: hardcodes shapes; builds one SPMD Bass program and runs it
via run_bass_kernel_spmd on cores 0-7.
"""
from contextlib import ExitStack

import numpy as np

import concourse.bass as bass
import concourse.tile as tile
from concourse import bacc, mybir
from concourse.bass_utils import run_bass_kernel_spmd
from concourse.masks import make_identity

F32 = mybir.dt.float32
F32R = mybir.dt.float32r

B, S, D, H, DH, HID = 4, 2048, 1024, 16, 64, 4096
SQ = S // 2           # query tokens per core
N_CORES = 8
LN_EPS = 1e-5
SCALE = 1.0 / np.sqrt(DH)

KO = D // 128         # 8   contraction subtiles over D
KT = S // 128         # 16  key-token tiles
QT = SQ // 128        # 8   query-token tiles
HP = H // 2           # 8   head pairs
HT = HID // 128       # 32  hidden tiles
HB = 4                # hidden blocks (of 8 ht = 1024 hid each)

_BUILD_CACHE = {}


def _build(flags, debug=None):
    """flags: frozenset of extras: bqkv, bo, b1, b2, g1b1, g2b2.
    debug: None | 'ctx' | 'y' (adds early outputs and stops there)."""
    nc = bacc.Bacc("TRN2", target_bir_lowering=False, debug=False)

    XT = nc.dram_tensor("XT", [D, S], F32R, kind="ExternalInput").ap()
    XQT = nc.dram_tensor("XQT", [D, SQ], F32R, kind="ExternalInput").ap()
    XQ = nc.dram_tensor("XQ", [SQ, D], F32, kind="ExternalInput").ap()
    WQ = nc.dram_tensor("WQ", [D, D], F32R, kind="ExternalInput").ap()
    WK = nc.dram_tensor("WK", [D, D], F32R, kind="ExternalInput").ap()
    WV = nc.dram_tensor("WV", [D, D], F32R, kind="ExternalInput").ap()
    WO = nc.dram_tensor("WO", [D, D], F32R, kind="ExternalInput").ap()
    W1 = nc.dram_tensor("W1", [D, HID], F32R, kind="ExternalInput").ap()
    W2 = nc.dram_tensor("W2", [HID, D], F32R, kind="ExternalInput").ap()
    OUT = nc.dram_tensor("OUT", [SQ, D], F32, kind="ExternalOutput").ap()
    if debug == "ctx":
        CTXD = nc.dram_tensor("CTXD", [128, HP, SQ], F32, kind="ExternalOutput").ap()
    if debug == "y":
        YD = nc.dram_tensor("YD", [QT, 128, D], F32, kind="ExternalOutput").ap()

    ext = {}
    if "bqkv" in flags:
        for nm in ("BQ", "BK", "BV"):
            ext[nm] = nc.dram_tensor(nm, [D], F32, kind="ExternalInput").ap()
    if "bo" in flags:
        ext["BO"] = nc.dram_tensor("BO", [D], F32, kind="ExternalInput").ap()
    if "b1" in flags:
        ext["B1"] = nc.dram_tensor("B1", [HID], F32, kind="ExternalInput").ap()
    if "b2" in flags:
        ext["B2"] = nc.dram_tensor("B2", [D], F32, kind="ExternalInput").ap()
    if "g1b1" in flags:
        ext["G1"] = nc.dram_tensor("G1", [D], F32, kind="ExternalInput").ap()
        ext["BT1"] = nc.dram_tensor("BT1", [D], F32, kind="ExternalInput").ap()
    if "g2b2" in flags:
        ext["G2"] = nc.dram_tensor("G2", [D], F32, kind="ExternalInput").ap()
        ext["BT2"] = nc.dram_tensor("BT2", [D], F32, kind="ExternalInput").ap()

    def bcast_free(vec_ap, parts):
        """1-D DRAM AP -> DMA source AP broadcast over `parts` partitions."""
        return bass.AP(tensor=vec_ap.tensor, offset=vec_ap.offset,
                       ap=[[0, parts]] + list(vec_ap.ap))

    WKr = WK.rearrange("(ko p) d -> p ko d", p=128)
    WQr = WQ.rearrange("(ko p) d -> p ko d", p=128)
    WVr = WV.rearrange("(ko p) d -> p ko d", p=128)
    WOr = WO.rearrange("(ko p) d -> p ko d", p=128)
    W1r = W1.rearrange("(ko p) h -> p ko h", p=128)
    W2r = W2.rearrange("(ho p) d -> p ho d", p=128)

    with tile.TileContext(nc) as tc, ExitStack() as ctx:
        persist = ctx.enter_context(tc.tile_pool(name="persist", bufs=1))
        dram = ctx.enter_context(tc.tile_pool(name="dram", bufs=1, space="DRAM"))

        Vd = dram.tile([KT, 128, H, DH + 1], F32R)
        KTd = dram.tile([HP, 128, S], F32R)
        QTd = dram.tile([HP, 128, SQ], F32R)
        Yd = dram.tile([QT, 128, D], F32)
        accD = dram.tile([QT, 128, D], F32)

        stk_x = ctx.enter_context(ExitStack())
        pX = stk_x.enter_context(tc.tile_pool(name="pX", bufs=1))
        xt = pX.tile([128, KO, S], F32R)
        nc.sync.dma_start(xt[:], XT.rearrange("(ko p) t -> p ko t", p=128))
        xqt = pX.tile([128, KO, SQ], F32R)
        nc.sync.dma_start(xqt[:], XQT.rearrange("(ko p) t -> p ko t", p=128))

        ones16_f = persist.tile([128, 16], F32)
        nc.vector.memset(ones16_f[:], 1.0)
        ones16 = persist.tile([128, 16], F32R)
        nc.scalar.copy(ones16[:], ones16_f[:])
        ones64_f = persist.tile([128, 64], F32)
        nc.vector.memset(ones64_f[:], 1.0)
        ones64 = persist.tile([128, 64], F32R)
        nc.scalar.copy(ones64[:], ones64_f[:])
        eps_sb = persist.tile([128, 1], F32)
        nc.vector.memset(eps_sb[:], LN_EPS)
        ident = persist.tile([128, 128], F32)
        make_identity(nc, ident[:])

        if "bqkv" in flags:
            bq_sb = persist.tile([128, KO], F32)
            bk_sb = persist.tile([128, KO], F32)
            nc.sync.dma_start(bq_sb[:], ext["BQ"].rearrange("(o p) -> p o", p=128))
            nc.sync.dma_start(bk_sb[:], ext["BK"].rearrange("(o p) -> p o", p=128))
        if "b1" in flags:
            b1_sb = persist.tile([128, HT], F32)
            nc.sync.dma_start(b1_sb[:], ext["B1"].rearrange("(o p) -> p o", p=128))
        if "bo" in flags:
            bo_sb = persist.tile([128, D], F32)
            nc.sync.dma_start(bo_sb[:], bcast_free(ext["BO"], 128))
        if "b2" in flags:
            b2_sb = persist.tile([128, D], F32)
            nc.sync.dma_start(b2_sb[:], bcast_free(ext["B2"], 128))
        if "g1b1" in flags:
            g1_sb = persist.tile([128, D], F32)
            bt1_sb = persist.tile([128, D], F32)
            nc.sync.dma_start(g1_sb[:], bcast_free(ext["G1"], 128))
            nc.sync.dma_start(bt1_sb[:], bcast_free(ext["BT1"], 128))
        if "g2b2" in flags:
            g2_sb = persist.tile([128, D], F32)
            bt2_sb = persist.tile([128, D], F32)
            nc.sync.dma_start(g2_sb[:], bcast_free(ext["G2"], 128))
            nc.sync.dma_start(bt2_sb[:], bcast_free(ext["BT2"], 128))

        # ---------- Phase A1: V projection -> Vd ----------
        with (
            tc.tile_pool(name="pA_w", bufs=2) as paw,
            tc.tile_pool(name="pA_s", bufs=3) as pas,
            tc.tile_pool(name="psA", bufs=3, space="PSUM") as psa,
        ):
            for dhalf in range(2):
                wv_h = paw.tile([128, KO, 512], F32R, tag="wv")
                nc.sync.dma_start(
                    wv_h[:], WVr[:, :, dhalf * 512:(dhalf + 1) * 512])
                for tt in range(KT):
                    pvp = psa.tile([128, 512], F32)
                    for k in range(KO):
                        nc.tensor.matmul(
                            pvp[:], xt[:, k, tt * 128:(tt + 1) * 128],
                            wv_h[:, k], start=(k == 0), stop=(k == KO - 1))
                    vstage = pas.tile([128, 8, DH], F32R, tag="vs")
                    vsv = vstage[:].rearrange("p a b -> p (a b)")
                    if "bqkv" in flags:
                        bvb = pas.tile([128, 512], F32, tag="bv")
                        nc.sync.dma_start(
                            bvb[:], bcast_free(
                                ext["BV"][dhalf * 512:(dhalf + 1) * 512], 128))
                        nc.vector.tensor_add(vsv, pvp[:], bvb[:])
                    else:
                        nc.scalar.copy(vsv, pvp[:])
                    nc.sync.dma_start(
                        Vd[tt, :, dhalf * 8:(dhalf + 1) * 8, 0:DH], vstage[:])
            for tt in range(KT):
                nc.sync.dma_start(Vd[tt, :, :, DH:DH + 1], ones16[:, :, None])

        # ---------- Phase A2: K^T / Q^T projections -> KTd / QTd ----------
        with (
            tc.tile_pool(name="pA2_w", bufs=2) as pa2w,
            tc.tile_pool(name="pA2_s", bufs=3) as pa2s,
            tc.tile_pool(name="psA2", bufs=3, space="PSUM") as psa2,
        ):
            for hp in range(HP):
                wk_hp = pa2w.tile([128, KO, 128], F32R, tag="wk")
                nc.sync.dma_start(wk_hp[:], WKr[:, :, hp * 128:(hp + 1) * 128])
                wq_hp = pa2w.tile([128, KO, 128], F32R, tag="wq")
                nc.sync.dma_start(wq_hp[:], WQr[:, :, hp * 128:(hp + 1) * 128])
                for ns in range(S // 512):
                    pk = psa2.tile([128, 512], F32)
                    for k in range(KO):
                        nc.tensor.matmul(
                            pk[:], wk_hp[:, k], xt[:, k, ns * 512:(ns + 1) * 512],
                            start=(k == 0), stop=(k == KO - 1))
                    ks = pa2s.tile([128, 512], F32R, tag="ks")
                    if "bqkv" in flags:
                        nc.scalar.activation(
                            ks[:], pk[:], mybir.ActivationFunctionType.Identity,
                            bias=bk_sb[:, hp:hp + 1])
                    else:
                        nc.scalar.copy(ks[:], pk[:])
                    nc.sync.dma_start(
                        KTd[hp, :, ns * 512:(ns + 1) * 512], ks[:])
                for ns in range(SQ // 512):
                    pq = psa2.tile([128, 512], F32)
                    for k in range(KO):
                        nc.tensor.matmul(
                            pq[:], wq_hp[:, k], xqt[:, k, ns * 512:(ns + 1) * 512],
                            start=(k == 0), stop=(k == KO - 1))
                    qs = pa2s.tile([128, 512], F32R, tag="qs")
                    if "bqkv" in flags:
                        nc.scalar.activation(
                            qs[:], pq[:], mybir.ActivationFunctionType.Identity,
                            bias=bq_sb[:, hp:hp + 1])
                    else:
                        nc.scalar.copy(qs[:], pq[:])
                    nc.sync.dma_start(
                        QTd[hp, :, ns * 512:(ns + 1) * 512], qs[:])

        stk_x.close()  # free xt/xqt

        # ---------- Phase B: attention per head-pair ----------
        # pYT opened before pCTX so closes can follow stack order
        # (pCTX closes after phase C, pYT at the end).
        stk_yt = ctx.enter_context(ExitStack())
        pYT = stk_yt.enter_context(tc.tile_pool(name="pYT", bufs=1))
        stk_ctx = ctx.enter_context(ExitStack())
        pCTX = stk_ctx.enter_context(tc.tile_pool(name="pCTX", bufs=1))
        ctxT = pCTX.tile([128, HP, SQ], F32R)
        # prefetch phase-C weights/residual during phase B
        stk_cw = ctx.enter_context(ExitStack())
        wo = xq = None
        if debug != "ctx":
            pcw = stk_cw.enter_context(tc.tile_pool(name="pC_w", bufs=1))
            pcx = stk_cw.enter_context(tc.tile_pool(name="pC_x", bufs=1))
            wo = pcw.tile([128, KO, D], F32R)
            nc.sync.dma_start(wo[:], WOr)
            xq = pcx.tile([128, QT, D], F32)
            nc.sync.dma_start(xq[:], XQ.rearrange("(qt p) d -> p qt d", p=128))
        with (
            tc.tile_pool(name="pB_kq", bufs=2) as pbkq,
            tc.tile_pool(name="pB_v", bufs=4) as pbv,
            tc.tile_pool(name="pB_p", bufs=6) as pbp,
            tc.tile_pool(name="pB_n", bufs=2) as pbn,
            tc.tile_pool(name="pB_st", bufs=5) as pbst,
            tc.tile_pool(name="psB", bufs=3, space="PSUM") as psb,
            tc.tile_pool(name="psPV", bufs=4, space="PSUM") as pspv,
            tc.tile_pool(name="psBC", bufs=1, space="PSUM") as psbc,
        ):
            for hp in range(HP):
                kt_hp = pbkq.tile([128, S], F32R, tag="kt")
                nc.sync.dma_start(kt_hp[:], KTd[hp])
                qt_hp = pbkq.tile([128, SQ], F32R, tag="qt")
                nc.sync.dma_start(qt_hp[:], QTd[hp])

                pv_ps = [[pspv.tile([128, 512], F32, tag="pv",
                                    name=f"pv_{hp}_{h}_{qb}")
                          for qb in range(2)]
                         for h in range(2)]  # [h][qb]
                for kt in range(KT):
                    v_kt = pbv.tile([128, 2, DH + 1], F32R, tag="v")
                    nc.sync.dma_start(v_kt[:], Vd[kt, :, 2 * hp:2 * hp + 2, :])
                    for qb in range(2):
                        for h in range(2):
                            ps_s = psb.tile([128, 512], F32, tag="ps_s")
                            nc.tensor.matmul(
                                ps_s[:],
                                kt_hp[h * 64:(h + 1) * 64, kt * 128:(kt + 1) * 128],
                                qt_hp[h * 64:(h + 1) * 64, qb * 512:(qb + 1) * 512],
                                start=True, stop=True)
                            p_t = pbp.tile([128, 512], F32R, tag="p")
                            nc.scalar.activation(
                                p_t[:], ps_s[:], mybir.ActivationFunctionType.Exp,
                                bias=0.0, scale=float(SCALE))
                            nc.tensor.matmul(
                                pv_ps[h][qb][0:DH + 1], v_kt[:, h], p_t[:],
                                start=(kt == 0), stop=(kt == KT - 1),
                                skip_group_check=True)
                # stage ctx+sumexp out of PSUM fast (frees the pv banks for
                # the next head-pair), then normalize from SBUF: broadcast
                # sumexp across 64 partitions via a ones-column matmul,
                # reciprocal, scale ctx rows
                stages = {}
                for qb in range(2):
                    for h in range(2):
                        stage = pbst.tile([128, 512], F32R, tag="stage",
                                          name=f"stg_{hp}_{h}_{qb}")
                        nc.scalar.copy(stage[0:DH + 1, :],
                                       pv_ps[h][qb][0:DH + 1, :])
                        stages[(h, qb)] = stage
                for qb in range(2):
                    for h in range(2):
                        stage = stages[(h, qb)]
                        bc = psbc.tile([64, 512], F32, tag="bc",
                                       name=f"bc_{hp}_{h}_{qb}")
                        nc.tensor.matmul(bc[:], ones64[64:65, :],
                                         stage[64:65, :], start=True, stop=True)
                        rb = pbn.tile([64, 512], F32, tag="rb",
                                      name=f"rb_{hp}_{h}_{qb}")
                        nc.vector.reciprocal(rb[:], bc[:])
                        if h == 0:
                            nc.vector.tensor_mul(
                                ctxT[0:64, hp, qb * 512:(qb + 1) * 512],
                                stage[0:DH], rb[:])
                        else:
                            tmp1 = pbn.tile([64, 512], F32R, tag="tmp1",
                                            name=f"tmp1_{hp}_{qb}")
                            nc.vector.tensor_mul(tmp1[:], stage[0:DH], rb[:])
                            nc.sync.dma_start(
                                ctxT[64:128, hp, qb * 512:(qb + 1) * 512],
                                tmp1[:])

        if debug == "ctx":
            with tc.tile_pool(name="dbg", bufs=2) as dbg:
                for hp in range(HP):
                    t = dbg.tile([128, SQ], F32)
                    nc.vector.tensor_copy(t[:], ctxT[:, hp, :])
                    nc.sync.dma_start(CTXD[:, hp, :], t[:])
            stk_ctx.close()
            out_stub = persist.tile([128, 1], F32)
            nc.vector.memset(out_stub[:], 0.0)
            nc.sync.dma_start(OUT[0:1, 0:128].rearrange("a b -> b a"), out_stub[:])

        phase_cd = debug != "ctx"
        phase_d = debug is None

        # ---------- Phase C: out-proj + LN1 + transpose ----------
        yt = pYT.tile([128, KO, SQ], F32R, name="yt") if phase_cd else None
        with (
            tc.tile_pool(name="pC_s", bufs=3) as pcs,
            tc.tile_pool(name="psC", bufs=3, space="PSUM") as psc,
            tc.tile_pool(name="psT", bufs=3, space="PSUM") as pst,
        ):
          if phase_cd:
            for qt in range(QT):
                r1 = pcs.tile([128, D], F32, tag="r1")
                for dh in range(2):
                    po = psc.tile([128, 512], F32)
                    for hp in range(HP):
                        nc.tensor.matmul(
                            po[:], ctxT[:, hp, qt * 128:(qt + 1) * 128],
                            wo[:, hp, dh * 512:(dh + 1) * 512],
                            start=(hp == 0), stop=(hp == HP - 1))
                    nc.vector.tensor_add(
                        r1[:, dh * 512:(dh + 1) * 512], po[:],
                        xq[:, qt, dh * 512:(dh + 1) * 512])
                if "bo" in flags:
                    nc.vector.tensor_add(r1[:], r1[:], bo_sb[:])
                stats = pcs.tile([128, 2, 6], F32, tag="st")
                r1v = r1[:].rearrange("p (s d) -> p s d", s=2)
                for sgi in range(2):
                    nc.vector.bn_stats(stats[:, sgi], r1v[:, sgi])
                mv = pcs.tile([128, 2], F32, tag="mv")
                nc.vector.bn_aggr(mv[:], stats[:])
                rstd = pcs.tile([128, 1], F32, tag="rstd")
                nc.scalar.activation(rstd[:], mv[:, 1:2],
                                     mybir.ActivationFunctionType.Sqrt,
                                     bias=eps_sb[:], scale=1.0)
                nc.vector.reciprocal(rstd[:], rstd[:])
                ytile = pcs.tile([128, D], F32, tag="ytile")
                nc.vector.tensor_scalar(
                    ytile[:], r1[:], scalar1=mv[:, 0:1], scalar2=rstd[:],
                    op0=mybir.AluOpType.subtract, op1=mybir.AluOpType.mult)
                if "g1b1" in flags:
                    nc.vector.tensor_mul(ytile[:], ytile[:], g1_sb[:])
                    nc.vector.tensor_add(ytile[:], ytile[:], bt1_sb[:])
                nc.sync.dma_start(Yd[qt], ytile[:])
                for dt in range(KO):
                    ptp = pst.tile([128, 128], F32)
                    nc.tensor.transpose(
                        ptp[:], ytile[:, dt * 128:(dt + 1) * 128], ident[:])
                    nc.vector.tensor_copy(
                        yt[:, dt, qt * 128:(qt + 1) * 128], ptp[:])

        if phase_cd:
            stk_cw.close()   # free wo/xq
            stk_ctx.close()  # free ctxT

        if debug == "y":
            with tc.tile_pool(name="dbg2", bufs=2) as dbg2:
                for qt in range(QT):
                    t = dbg2.tile([128, D], F32)
                    nc.sync.dma_start(t[:], Yd[qt])
                    nc.sync.dma_start(YD[qt], t[:])
            out_stub2 = persist.tile([128, 1], F32)
            nc.vector.memset(out_stub2[:], 0.0)
            nc.sync.dma_start(OUT[0:1, 0:128].rearrange("a b -> b a"), out_stub2[:])

        # ---------- Phase D: FFN + LN2 + output ----------
        with (
            tc.tile_pool(name="pD_w1", bufs=2) as pw1,
            tc.tile_pool(name="pD_w2", bufs=1) as pw2,
            tc.tile_pool(name="pD_ft", bufs=1) as pft,
            tc.tile_pool(name="pD_s", bufs=2) as pds,
            tc.tile_pool(name="psD", bufs=3, space="PSUM") as psd,
            tc.tile_pool(name="psD2", bufs=4, space="PSUM") as psd2,
        ):
          if phase_d:
            for hb in range(HB):
                w1_hb = pw1.tile([128, KO, 1024], F32R)
                nc.sync.dma_start(
                    w1_hb[:], W1r[:, :, hb * 1024:(hb + 1) * 1024])
                w2_hb = pw2.tile([128, 8, D], F32R)
                nc.sync.dma_start(w2_hb[:], W2r[:, hb * 8:(hb + 1) * 8, :])
                ft = pft.tile([128, 8, SQ], F32R)
                for hti in range(8):
                    for qb in range(2):
                        pf = psd.tile([128, 512], F32)
                        for k in range(KO):
                            nc.tensor.matmul(
                                pf[:], w1_hb[:, k, hti * 128:(hti + 1) * 128],
                                yt[:, k, qb * 512:(qb + 1) * 512],
                                start=(k == 0), stop=(k == KO - 1))
                        if "b1" in flags:
                            nc.scalar.activation(
                                ft[:, hti, qb * 512:(qb + 1) * 512], pf[:],
                                mybir.ActivationFunctionType.Relu,
                                bias=b1_sb[:, hb * 8 + hti:hb * 8 + hti + 1])
                        else:
                            nc.scalar.activation(
                                ft[:, hti, qb * 512:(qb + 1) * 512], pf[:],
                                mybir.ActivationFunctionType.Relu)
                for qt in range(QT):
                    for dh in range(2):
                        p2 = psd2.tile([128, 512], F32)
                        for hti in range(8):
                            nc.tensor.matmul(
                                p2[:], ft[:, hti, qt * 128:(qt + 1) * 128],
                                w2_hb[:, hti, dh * 512:(dh + 1) * 512],
                                start=(hti == 0), stop=(hti == 7))
                        f2s = pds.tile([128, 512], F32, tag="f2s",
                                       name=f"f2s_{hb}_{qt}_{dh}")
                        nc.vector.tensor_copy(f2s[:], p2[:])
                        nc.gpsimd.dma_start(
                            accD[qt, :, dh * 512:(dh + 1) * 512], f2s[:],
                            accum_op=(mybir.AluOpType.bypass if hb == 0
                                      else mybir.AluOpType.add))
            # residual + LN2 + store
            for qt in range(QT):
                yr = pds.tile([128, D], F32, tag="yr")
                nc.sync.dma_start(yr[:], Yd[qt])
                ac = pds.tile([128, D], F32, tag="ac")
                nc.sync.dma_start(ac[:], accD[qt])
                r2 = pds.tile([128, D], F32, tag="r2")
                nc.vector.tensor_add(r2[:], ac[:], yr[:])
                if "b2" in flags:
                    nc.vector.tensor_add(r2[:], r2[:], b2_sb[:])
                stats = pds.tile([128, 2, 6], F32, tag="st2")
                r2v = r2[:].rearrange("p (s d) -> p s d", s=2)
                for sgi in range(2):
                    nc.vector.bn_stats(stats[:, sgi], r2v[:, sgi])
                mv = pds.tile([128, 2], F32, tag="mv2")
                nc.vector.bn_aggr(mv[:], stats[:])
                rstd = pds.tile([128, 1], F32, tag="rstd2")
                nc.scalar.activation(rstd[:], mv[:, 1:2],
                                     mybir.ActivationFunctionType.Sqrt,
                                     bias=eps_sb[:], scale=1.0)
                nc.vector.reciprocal(rstd[:], rstd[:])
                o = pds.tile([128, D], F32, tag="o")
                nc.vector.tensor_scalar(
                    o[:], r2[:], scalar1=mv[:, 0:1], scalar2=rstd[:],
                    op0=mybir.AluOpType.subtract, op1=mybir.AluOpType.mult)
                if "g2b2" in flags:
                    nc.vector.tensor_mul(o[:], o[:], g2_sb[:])
                    nc.vector.tensor_add(o[:], o[:], bt2_sb[:])
                nc.sync.dma_start(
                    OUT.rearrange("(qt p) d -> qt p d", p=128)[qt], o[:])

    nc.compile()
    return nc


def _get_program(flags, debug=None):
    key = (flags, debug)
    if key not in _BUILD_CACHE:
        _BUILD_CACHE[key] = _build(flags, debug)
    return _BUILD_CACHE[key]


def _make_in_maps(X, shared):
    in_maps = []
    for c in range(N_CORES):
        b, half = c // 2, c % 2
        xq = np.ascontiguousarray(X[b, half * SQ:(half + 1) * SQ])
        m = dict(shared)
        m.update({"XT": np.ascontiguousarray(X[b].T),
                  "XQT": np.ascontiguousarray(xq.T), "XQ": xq})
        in_maps.append(m)
    return in_maps


def kernel(X, Wq, bq, Wk, bk, Wv, bv, Wo, bo, g1, beta1, W1, b1, W2, b2, g2,
           beta2, _debug=None, _trace=False):
    f32 = lambda a: np.ascontiguousarray(np.asarray(a), dtype=np.float32)
    X = f32(X)
    Wq, Wk, Wv, Wo, W1, W2 = map(f32, (Wq, Wk, Wv, Wo, W1, W2))
    bq, bk, bv, bo, b1, b2 = map(f32, (bq, bk, bv, bo, b1, b2))
    g1, beta1, g2, beta2 = map(f32, (g1, beta1, g2, beta2))

    flags = set()
    if bq.any() or bk.any() or bv.any():
        flags.add("bqkv")
    if bo.any():
        flags.add("bo")
    if b1.any():
        flags.add("b1")
    if b2.any():
        flags.add("b2")
    if (g1 != 1).any() or beta1.any():
        flags.add("g1b1")
    if (g2 != 1).any() or beta2.any():
        flags.add("g2b2")
    flags = frozenset(flags)

    nc = _get_program(flags, _debug)

    shared = {"WQ": Wq, "WK": Wk, "WV": Wv, "WO": Wo, "W1": W1, "W2": W2}
    if "bqkv" in flags:
        shared.update({"BQ": bq, "BK": bk, "BV": bv})
    if "bo" in flags:
        shared["BO"] = bo
    if "b1" in flags:
        shared["B1"] = b1
    if "b2" in flags:
        shared["B2"] = b2
    if "g1b1" in flags:
        shared.update({"G1": g1, "BT1": beta1})
    if "g2b2" in flags:
        shared.update({"G2": g2, "BT2": beta2})

    in_maps = _make_in_maps(X, shared)
    res = run_bass_kernel_spmd(nc, in_maps, core_ids=list(range(N_CORES)),
                               trace=_trace)

    if _debug is not None or _trace:
        return res

    out = np.empty((B, S, D), dtype=np.float32)
    for c in range(N_CORES):
        b, half = c // 2, c % 2
        out[b, half * SQ:(half + 1) * SQ] = res.results[c]["OUT"]
    return out
